# revision 8
# baseline (speedup 1.0000x reference)
"""Trainium2 Bass kernel: batched locally-weighted ridge regression.

Per test point t: K[t,n] = exp(-|xte_t - xtr_n|^2 / (2 ls^2));
  A_t = Xtild^T diag(K[t]) Xtild + REG*I ; b_t = Xtild^T (K[t] * Y)
  ypred_t = xtild_t . A_t^{-1} b_t
Sharding: data-parallel over the 4096 test points -> 8 cores x 512.

On-device math uses a scaled kernel K'[t,n] = exp((S[n,t] - sn[n]/2) * c2)
(c2 = 1/ls^2), i.e. the exp(-st*c2/2) per-test factor is dropped; this
rescales A_t and b_t identically, so beta is preserved by using a
per-test ridge REG_t = REG * exp(st*c2/2).

v2 layout:
  - Host precomputes the outer-product expansion Z = [x_d*x_e | x*y]
    (train-only data), squared-norm exp biases, per-test ridge, and the
    test design rows; DMA'd in parallel streams.
  - PE: warmup matmuls (clock ramp), gram S = XtrT-groups @ XteT in
    f32r, then XWX/XWy accumulation K'-chunks @ Z in f32r.
  - ACT: exp(S*c2 + bias), PSUM evacuations into [A|b] systems, mirror.
  - DVE+Pool: batched Gaussian elimination, 2 halves of 2x128 systems,
    rows of each update split DVE (top) / Pool (bottom); back-subst
    and prediction dot products on Pool; reductions/reciprocals on DVE.
"""

import numpy as np

import concourse.bacc as bacc
import concourse.mybir as mybir
from concourse.bass import ds, ts
from concourse.bass_utils import run_bass_kernel_spmd
from concourse.tile import TileContext

F32 = mybir.dt.float32
F32R = mybir.dt.float32r
P = 128
N_TRAIN = 2048
D = 31
DP = 32          # 1 + D
W = 33           # DP + rhs column
N_TEST = 4096
NCORES = 8
TS = N_TEST // NCORES   # 512 test points per core
NT = TS // P            # 4 t-tiles
NK = N_TRAIN // P       # 16 train chunks
NZ = 800                # 512 (d x e>=16) + 256 (d<16 x e<16) + 32 (x*y)
REG = 1e-6
H = 16

N_WARMUP = 4            # PE clock-ramp warmup matmuls (fp32, 4-pass)
POOL_FRAC = 0.63        # fraction of elimination rows on the Pool engine


def _pool_rows(m: int) -> int:
    if m <= 3:
        return 0
    return min(m - 1, int(m * POOL_FRAC + 0.5))


def _build_nc(c2: float):
    """Build the single-core Bass program (SPMD across 8 cores)."""
    nc = bacc.Bacc(trn_type="TRN2")

    # transposed features packed for 4-way row-group gram matmuls:
    # [32g+d, cc*128+p] = Xtrain[(4*cc+g)*128+p, d]; cols 512: = XtestT x4
    xT_d = nc.dram_tensor("xT", [P, 4 * P + TS], F32R, kind="ExternalInput")
    zz_d = nc.dram_tensor("zz", [P, NK * NZ], F32R, kind="ExternalInput")
    bias_d = nc.dram_tensor("bias_n", [P, NK], F32, kind="ExternalInput")
    regt_d = nc.dram_tensor("regt", [P, NT], F32, kind="ExternalInput")
    xtt_d = nc.dram_tensor("xtt", [P, NT * DP], F32, kind="ExternalInput")
    out_d = nc.dram_tensor("ypred", [TS, 1], F32, kind="ExternalOutput")

    with TileContext(nc) as tc:
        with (
            tc.tile_pool(name="sb", bufs=1) as sb,
            tc.tile_pool(name="pwu", bufs=1, space="PSUM") as pwu,
            tc.tile_pool(name="pgram", bufs=3, space="PSUM") as pgram,
            tc.tile_pool(name="pxwx", bufs=4, space="PSUM") as pxwx,
        ):
            # ---- input loads; zz split across queues for parallel DMA ----
            xT = sb.tile([P, 4 * P + TS], F32R)
            nc.sync.dma_start(xT[:, 0:512], xT_d[:, 0:512])
            nc.sync.dma_start(xT[:, 512:], xT_d[:, 512:])
            zz = sb.tile([P, NK, NZ], F32R)
            zr = zz[:].rearrange("p c z -> p (c z)")
            for q in range(8):
                nc.sync.dma_start(
                    zr[:, ts(q, 2 * NZ)], zz_d[:, ts(q, 2 * NZ)]
                )
            bias_n = sb.tile([P, NK], F32)
            nc.sync.dma_start(bias_n, bias_d[:, :])
            regt = sb.tile([P, NT], F32)
            nc.sync.dma_start(regt, regt_d[:, :])
            xtt = sb.tile([P, NT, DP], F32)
            nc.sync.dma_start(
                xtt, xtt_d.rearrange("p (t d) -> p t d", t=NT)
            )

            # ---- PE warmup: ramp the clock during the DMAs ----
            wu = sb.tile([P, 256], F32)
            nc.vector.memset(wu, 1.0)
            for _ in range(N_WARMUP):
                wps = pwu.tile([P, 256], F32, tag="wu")
                nc.tensor.matmul(wps, wu[0:8, 0:128], wu[0:8, :],
                                 start=True, stop=True)

            # ---- gram + K' = exp(S*c2 - sn*c2/2), layout [n_chunk, t] ----
            kp = sb.tile([P, NK, TS], F32R)
            for cc in range(NK // 4):
                for g in range(4):
                    c = 4 * cc + g
                    sg = pgram.tile([P, TS], F32, tag="sg")
                    nc.tensor.matmul(
                        sg,
                        xT[32 * g:32 * g + D, ts(cc, P)],
                        xT[32 * g:32 * g + D, 4 * P:],
                        start=True, stop=True,
                        tile_position=(32 * g, 0),
                    )
                    nc.scalar.activation(
                        kp[:, c, :], sg, mybir.ActivationFunctionType.Exp,
                        bias=bias_n[:, ds(c, 1)], scale=c2,
                    )

            # ---- per-half: XWX/XWy matmuls, assembly, solve, predict ----
            ga = sb.tile([P, NT, DP, W], F32)
            invp = sb.tile([P, NT, DP], F32)
            xsol = sb.tile([P, NT, DP], F32)
            yp = sb.tile([P, NT], F32)
            fbD = [sb.tile([P, 2, D], F32, name=f"fbD{i}")
                   for i in range(2)]
            tbD = [sb.tile([P, 2, D, DP], F32, name=f"tbD{i}")
                   for i in range(2)]
            fbP = [sb.tile([P, 2, D], F32, name=f"fbP{i}")
                   for i in range(2)]
            tbP = [sb.tile([P, 2, D, DP], F32, name=f"tbP{i}")
                   for i in range(2)]
            bsc = [sb.tile([P, 2, D], F32, name=f"bsc{i}")
                   for i in range(2)]
            prod = [sb.tile([P, 2, DP], F32, name=f"prod{i}")
                   for i in range(2)]
            ga_sw = ga[:].rearrange("p b r c -> p b c r")
            ga_diag = ga[:].rearrange("p b r c -> p b (r c)")[:, :, ::W + 1]

            for h in range(2):
                b0, b1 = 2 * h, 2 * h + 2
                # XWX | XWy: [256, 1056] per half via K'-chunks @ Z
                for t in range(b0, b1):
                    for (c0, c1) in ((0, 512), (512, NZ)):
                        w = c1 - c0
                        px = pxwx.tile([P, 512], F32, tag="px")
                        for c in range(NK):
                            nc.tensor.matmul(
                                px[:, :w],
                                kp[:, c, ts(t, P)],
                                zz[:, c, c0:c1],
                                start=(c == 0), stop=(c == NK - 1),
                            )
                        if c0 == 0:
                            # cols e=16..31, all rows d
                            nc.scalar.copy(
                                ga[:, t, :, H:DP],
                                px[:, :w].rearrange("p (r c) -> p r c", r=DP),
                            )
                        else:
                            # top-left quadrant + rhs column
                            nc.scalar.copy(
                                ga[:, t, 0:H, 0:H],
                                px[:, 0:H * H].rearrange(
                                    "p (r c) -> p r c", r=H),
                            )
                            nc.scalar.copy(
                                ga[:, t, :, DP], px[:, H * H:H * H + DP])

                # mirror lower-left quadrant from upper-right (A symmetric)
                nc.scalar.copy(
                    ga[:, b0:b1, H:DP, 0:H],
                    ga_sw[:, b0:b1, H:DP, 0:H],
                )
                # per-test ridge on the diagonal
                nc.gpsimd.tensor_add(
                    ga_diag[:, b0:b1], ga_diag[:, b0:b1],
                    regt[:, b0:b1, None].broadcast_to([P, 2, DP]),
                )

                # ---- batched Gaussian elimination (no pivoting; A SPD),
                # update rows split: DVE takes the top mD, Pool bottom mP
                for k in range(DP):
                    nc.vector.reciprocal(
                        invp[:, b0:b1, k], ga[:, b0:b1, k, k])
                    if k == DP - 1:
                        break
                    m = D - k
                    w = DP - k
                    mP = _pool_rows(m)
                    mD = m - mP
                    prow = ga[:, b0:b1, k:k + 1, k + 1:W]
                    iv = invp[:, b0:b1, k:k + 1]
                    r0 = k + 1
                    nc.vector.tensor_mul(
                        fbD[h][:, :, :mD],
                        ga[:, b0:b1, r0:r0 + mD, k],
                        iv.broadcast_to([P, 2, mD]),
                    )
                    nc.vector.tensor_mul(
                        tbD[h][:, :, :mD, :w],
                        fbD[h][:, :, :mD, None].broadcast_to([P, 2, mD, w]),
                        prow.broadcast_to([P, 2, mD, w]),
                    )
                    nc.vector.tensor_sub(
                        ga[:, b0:b1, r0:r0 + mD, k + 1:W],
                        ga[:, b0:b1, r0:r0 + mD, k + 1:W],
                        tbD[h][:, :, :mD, :w],
                    )
                    if mP:
                        r1 = r0 + mD
                        nc.gpsimd.tensor_mul(
                            fbP[h][:, :, :mP],
                            ga[:, b0:b1, r1:r1 + mP, k],
                            iv.broadcast_to([P, 2, mP]),
                        )
                        nc.gpsimd.tensor_mul(
                            tbP[h][:, :, :mP, :w],
                            fbP[h][:, :, :mP, None].broadcast_to(
                                [P, 2, mP, w]),
                            prow.broadcast_to([P, 2, mP, w]),
                        )
                        nc.gpsimd.tensor_sub(
                            ga[:, b0:b1, r1:r1 + mP, k + 1:W],
                            ga[:, b0:b1, r1:r1 + mP, k + 1:W],
                            tbP[h][:, :, :mP, :w],
                        )

                # ---- backward substitution on the rhs column (Pool) ----
                for k in range(DP - 1, -1, -1):
                    nc.gpsimd.tensor_mul(
                        xsol[:, b0:b1, k], ga[:, b0:b1, k, DP],
                        invp[:, b0:b1, k],
                    )
                    if k == 0:
                        break
                    nc.gpsimd.tensor_mul(
                        bsc[h][:, :, :k],
                        ga[:, b0:b1, 0:k, k],
                        xsol[:, b0:b1, k:k + 1].broadcast_to([P, 2, k]),
                    )
                    nc.gpsimd.tensor_sub(
                        ga[:, b0:b1, 0:k, DP], ga[:, b0:b1, 0:k, DP],
                        bsc[h][:, :, :k],
                    )

                # ---- predictions: ypred = xtild_test . beta ----
                nc.gpsimd.tensor_mul(
                    prod[h], xtt[:, b0:b1], xsol[:, b0:b1])
                nc.vector.tensor_reduce(
                    yp[:, b0:b1], prod[h],
                    mybir.AxisListType.X, mybir.AluOpType.add,
                )

            nc.sync.dma_start(
                out_d.rearrange("(t p) one -> p (t one)", p=P), yp
            )

    nc.finalize()
    return nc


_cache: dict[float, object] = {}


def _get_nc(c2: float):
    if c2 not in _cache:
        _cache[c2] = _build_nc(c2)
    return _cache[c2]


def _build_xT(Xtrain, shard):
    """Pack [XtrT | XteT] with chunks at partition offsets 32g for 4-way
    row-group gram matmuls."""
    out = np.zeros((P, 4 * P + TS), np.float32)
    XtrT = Xtrain.T
    for g in range(4):
        for cc in range(4):
            c = 4 * cc + g
            out[32 * g:32 * g + D, cc * P:(cc + 1) * P] = \
                XtrT[:, c * P:(c + 1) * P]
        out[32 * g:32 * g + D, 4 * P:] = shard.T
    return out


def _host_pack(Ytrain, Xtrain, c2):
    """Train-side packing shared by all cores: Z expansion + exp biases."""
    Xt = np.concatenate(
        [np.ones((N_TRAIN, 1), np.float32), Xtrain], axis=1)  # [2048, 32]
    A = (Xt[:, :, None] * Xt[:, None, H:DP]).reshape(N_TRAIN, DP * H)
    B = (Xt[:, :H, None] * Xt[:, None, :H]).reshape(N_TRAIN, H * H)
    C = Xt * Ytrain[:, 0:1]
    zz = np.concatenate([A, B, C], axis=1)              # [2048, 800]
    zz = np.ascontiguousarray(
        zz.reshape(NK, P, NZ).transpose(1, 0, 2).reshape(P, NK * NZ))
    sn = np.sum(Xtrain * Xtrain, axis=1)                # [2048]
    bias_n = np.ascontiguousarray(
        (-0.5 * c2 * sn).reshape(NK, P).T.astype(np.float32))
    return zz, bias_n


def _host_pack_test(shard, c2):
    """Test-side packing per core: ridge scale + design rows."""
    st = np.sum(shard * shard, axis=1)                  # [512]
    regt = np.ascontiguousarray(
        (REG * np.exp(0.5 * c2 * st)).reshape(NT, P).T.astype(np.float32))
    xtt = np.concatenate(
        [np.ones((TS, 1), np.float32), shard], axis=1)  # [512, 32]
    xtt = np.ascontiguousarray(
        xtt.reshape(NT, P, DP).transpose(1, 0, 2).reshape(P, NT * DP))
    return regt, xtt


def kernel(Ytrain, Xtrain, Xtest, log_lengthscale, _trace=False):
    Ytrain = np.ascontiguousarray(np.asarray(Ytrain, dtype=np.float32))
    Xtrain = np.ascontiguousarray(np.asarray(Xtrain, dtype=np.float32))
    Xtest = np.ascontiguousarray(np.asarray(Xtest, dtype=np.float32))
    lls = float(np.asarray(log_lengthscale, dtype=np.float32))
    c2 = float(np.exp(np.float32(-2.0 * lls)))

    nc = _get_nc(c2)
    zz, bias_n = _host_pack(Ytrain, Xtrain, c2)
    in_maps = []
    for core in range(NCORES):
        shard = np.ascontiguousarray(Xtest[core * TS:(core + 1) * TS])
        regt, xtt = _host_pack_test(shard, c2)
        in_maps.append({
            "xT": _build_xT(Xtrain, shard),
            "zz": zz,
            "bias_n": bias_n,
            "regt": regt,
            "xtt": xtt,
        })
    res = run_bass_kernel_spmd(nc, in_maps, list(range(NCORES)),
                               trace=bool(_trace))
    outs = [np.asarray(res.results[c]["ypred"], dtype=np.float32)
            for c in range(NCORES)]
    full = np.concatenate(outs, axis=0)
    if _trace:
        return full, res
    return full


def _sim_in_map(inputs):
    """Core-0 input map for CoreSim timing (test.py helper)."""
    Ytrain = np.asarray(inputs["Ytrain"], dtype=np.float32)
    Xtrain = np.asarray(inputs["Xtrain"], dtype=np.float32)
    Xtest = np.asarray(inputs["Xtest"], dtype=np.float32)
    lls = float(np.asarray(inputs["log_lengthscale"], dtype=np.float32))
    c2 = float(np.exp(np.float32(-2.0 * lls)))
    shard = np.ascontiguousarray(Xtest[:TS])
    zz, bias_n = _host_pack(Ytrain, Xtrain, c2)
    regt, xtt = _host_pack_test(shard, c2)
    return c2, {
        "xT": _build_xT(Xtrain, shard),
        "zz": zz,
        "bias_n": bias_n,
        "regt": regt,
        "xtt": xtt,
    }


# revision 10
# speedup vs baseline: 1.1896x; 1.1896x over previous
"""Trainium2 Bass kernel: batched locally-weighted ridge regression.

Per test point t: K[t,n] = exp(-|xte_t - xtr_n|^2 / (2 ls^2));
  A_t = Xtild^T diag(K[t]) Xtild + REG*I ; b_t = Xtild^T (K[t] * Y)
  ypred_t = xtild_t . A_t^{-1} b_t
Sharding: data-parallel over the 4096 test points -> 8 cores x 512.

On-device math uses a scaled kernel K'[t,n] = exp((S[n,t] - sn[n]/2) * c2)
(c2 = 1/ls^2), i.e. the exp(-st*c2/2) per-test factor is dropped; this
rescales A_t and b_t identically, so beta is preserved by using a
per-test ridge REG_t = REG * exp(st*c2/2).

v2 layout:
  - Host precomputes the outer-product expansion Z = [x_d*x_e | x*y]
    (train-only data), squared-norm exp biases, per-test ridge, and the
    test design rows; DMA'd in parallel streams.
  - PE: warmup matmuls (clock ramp), gram S = XtrT-groups @ XteT in
    f32r, then XWX/XWy accumulation K'-chunks @ Z in f32r.
  - ACT: exp(S*c2 + bias), PSUM evacuations into [A|b] systems, mirror.
  - DVE+Pool: batched Gaussian elimination, 2 halves of 2x128 systems,
    rows of each update split DVE (top) / Pool (bottom); back-subst
    and prediction dot products on Pool; reductions/reciprocals on DVE.
"""

import numpy as np

import concourse.bacc as bacc
import concourse.mybir as mybir
from concourse.bass import ds, ts
from concourse.bass_utils import run_bass_kernel_spmd
from concourse.tile import TileContext

F32 = mybir.dt.float32
F32R = mybir.dt.float32r
P = 128
N_TRAIN = 2048
D = 31
DP = 32          # 1 + D
W = 33           # DP + rhs column
N_TEST = 4096
NCORES = 8
TS = N_TEST // NCORES   # 512 test points per core
NT = TS // P            # 4 t-tiles
NK = N_TRAIN // P       # 16 train chunks
NZ = 800                # 512 (d x e>=16) + 256 (d<16 x e<16) + 32 (x*y)
REG = 1e-6
H = 16

N_WARMUP = 4            # PE clock-ramp warmup matmuls (fp32, 4-pass)
POOL_FRAC = 0.63        # fraction of elimination rows on the Pool engine


def _pool_rows(m: int) -> int:
    if m <= 3:
        return 0
    return min(m - 1, int(m * POOL_FRAC + 0.5))


def _build_nc(c2: float):
    """Build the single-core Bass program (SPMD across 8 cores)."""
    nc = bacc.Bacc(trn_type="TRN2")

    # transposed features packed for 4-way row-group gram matmuls:
    # [32g+d, cc*128+p] = Xtrain[(4*cc+g)*128+p, d]; cols 512: = XtestT x4
    xT_d = nc.dram_tensor("xT", [P, 4 * P + TS], F32R, kind="ExternalInput")
    zz_d = nc.dram_tensor("zz", [P, NK * NZ], F32R, kind="ExternalInput")
    bias_d = nc.dram_tensor("bias_n", [P, NK], F32, kind="ExternalInput")
    regt_d = nc.dram_tensor("regt", [P, NT], F32, kind="ExternalInput")
    xtt_d = nc.dram_tensor("xtt", [P, NT * DP], F32, kind="ExternalInput")
    out_d = nc.dram_tensor("ypred", [TS, 1], F32, kind="ExternalOutput")

    with TileContext(nc) as tc:
        with (
            tc.tile_pool(name="sb", bufs=1) as sb,
            tc.tile_pool(name="pwu", bufs=1, space="PSUM") as pwu,
            tc.tile_pool(name="pgram", bufs=3, space="PSUM") as pgram,
            tc.tile_pool(name="pxwx", bufs=4, space="PSUM") as pxwx,
        ):
            # ---- input loads; zz spread across engine DMA queues so the
            # transfers run in parallel (issuing engine is charged the
            # transfer time in the DGE model)
            xT = sb.tile([P, 4 * P + TS], F32R)
            nc.sync.dma_start(xT[:, 0:512], xT_d[:, 0:512])
            nc.sync.dma_start(xT[:, 512:], xT_d[:, 512:])
            bias_n = sb.tile([P, NK], F32)
            nc.sync.dma_start(bias_n, bias_d[:, :])
            regt = sb.tile([P, NT], F32)
            nc.sync.dma_start(regt, regt_d[:, :])
            xtt = sb.tile([P, NT, DP], F32)
            nc.sync.dma_start(
                xtt, xtt_d.rearrange("p (t d) -> p t d", t=NT)
            )
            zz = sb.tile([P, NK, NZ], F32R)
            zr = zz[:].rearrange("p c z -> p (c z)")
            zq = [nc.sync, nc.sync, nc.sync, nc.scalar, nc.scalar,
                  nc.scalar, nc.gpsimd, nc.gpsimd]
            for q in range(8):
                zq[q].dma_start(
                    zr[:, ts(q, 2 * NZ)], zz_d[:, ts(q, 2 * NZ)]
                )

            # ---- PE warmup: ramp the clock during the DMAs ----
            wu = sb.tile([P, 256], F32)
            nc.vector.memset(wu, 1.0)
            for _ in range(N_WARMUP):
                wps = pwu.tile([P, 256], F32, tag="wu")
                nc.tensor.matmul(wps, wu[0:8, 0:128], wu[0:8, :],
                                 start=True, stop=True)

            # ---- gram + K' = exp(S*c2 - sn*c2/2), layout [n_chunk, t] ----
            kp = sb.tile([P, NK, TS], F32R)
            for cc in range(NK // 4):
                for g in range(4):
                    c = 4 * cc + g
                    sg = pgram.tile([P, TS], F32, tag="sg")
                    nc.tensor.matmul(
                        sg,
                        xT[32 * g:32 * g + D, ts(cc, P)],
                        xT[32 * g:32 * g + D, 4 * P:],
                        start=True, stop=True,
                        tile_position=(32 * g, 0),
                    )
                    nc.scalar.activation(
                        kp[:, c, :], sg, mybir.ActivationFunctionType.Exp,
                        bias=bias_n[:, ds(c, 1)], scale=c2,
                    )

            # ---- per-half: XWX/XWy matmuls, assembly, solve, predict ----
            ga = sb.tile([P, NT, DP, W], F32)
            invp = sb.tile([P, NT, DP], F32)
            xsol = sb.tile([P, NT, DP], F32)
            yp = sb.tile([P, NT], F32)
            fbD = [sb.tile([P, 2, D], F32, name=f"fbD{i}")
                   for i in range(2)]
            tbD = [sb.tile([P, 2, D, DP], F32, name=f"tbD{i}")
                   for i in range(2)]
            fbP = [sb.tile([P, 2, D], F32, name=f"fbP{i}")
                   for i in range(2)]
            tbP = [sb.tile([P, 2, D, DP], F32, name=f"tbP{i}")
                   for i in range(2)]
            bsc = [sb.tile([P, 2, D], F32, name=f"bsc{i}")
                   for i in range(2)]
            prod = [sb.tile([P, 2, DP], F32, name=f"prod{i}")
                   for i in range(2)]
            ga_sw = ga[:].rearrange("p b r c -> p b c r")
            ga_diag = ga[:].rearrange("p b r c -> p b (r c)")[:, :, ::W + 1]

            for h in range(2):
                b0, b1 = 2 * h, 2 * h + 2
                # XWX | XWy: [256, 1056] per half via K'-chunks @ Z
                for t in range(b0, b1):
                    for (c0, c1) in ((0, 512), (512, NZ)):
                        w = c1 - c0
                        px = pxwx.tile([P, 512], F32, tag="px")
                        for c in range(NK):
                            nc.tensor.matmul(
                                px[:, :w],
                                kp[:, c, ts(t, P)],
                                zz[:, c, c0:c1],
                                start=(c == 0), stop=(c == NK - 1),
                            )
                        if c0 == 0:
                            # cols e=16..31, all rows d
                            nc.scalar.copy(
                                ga[:, t, :, H:DP],
                                px[:, :w].rearrange("p (r c) -> p r c", r=DP),
                            )
                        else:
                            # top-left quadrant + rhs column
                            nc.scalar.copy(
                                ga[:, t, 0:H, 0:H],
                                px[:, 0:H * H].rearrange(
                                    "p (r c) -> p r c", r=H),
                            )
                            nc.scalar.copy(
                                ga[:, t, :, DP], px[:, H * H:H * H + DP])

                # mirror lower-left quadrant from upper-right (A symmetric)
                nc.scalar.copy(
                    ga[:, b0:b1, H:DP, 0:H],
                    ga_sw[:, b0:b1, H:DP, 0:H],
                )
                # per-test ridge on the diagonal
                nc.gpsimd.tensor_add(
                    ga_diag[:, b0:b1], ga_diag[:, b0:b1],
                    regt[:, b0:b1, None].broadcast_to([P, 2, DP]),
                )

                # ---- batched Gaussian elimination (no pivoting; A SPD),
                # update rows split: DVE takes the top mD, Pool bottom mP
                for k in range(DP):
                    nc.vector.reciprocal(
                        invp[:, b0:b1, k], ga[:, b0:b1, k, k])
                    if k == DP - 1:
                        break
                    m = D - k
                    w = DP - k
                    mP = _pool_rows(m)
                    mD = m - mP
                    prow = ga[:, b0:b1, k:k + 1, k + 1:W]
                    iv = invp[:, b0:b1, k:k + 1]
                    r0 = k + 1
                    nc.vector.tensor_mul(
                        fbD[h][:, :, :mD],
                        ga[:, b0:b1, r0:r0 + mD, k],
                        iv.broadcast_to([P, 2, mD]),
                    )
                    nc.vector.tensor_mul(
                        tbD[h][:, :, :mD, :w],
                        fbD[h][:, :, :mD, None].broadcast_to([P, 2, mD, w]),
                        prow.broadcast_to([P, 2, mD, w]),
                    )
                    nc.vector.tensor_sub(
                        ga[:, b0:b1, r0:r0 + mD, k + 1:W],
                        ga[:, b0:b1, r0:r0 + mD, k + 1:W],
                        tbD[h][:, :, :mD, :w],
                    )
                    if mP:
                        r1 = r0 + mD
                        nc.gpsimd.tensor_mul(
                            fbP[h][:, :, :mP],
                            ga[:, b0:b1, r1:r1 + mP, k],
                            iv.broadcast_to([P, 2, mP]),
                        )
                        nc.gpsimd.tensor_mul(
                            tbP[h][:, :, :mP, :w],
                            fbP[h][:, :, :mP, None].broadcast_to(
                                [P, 2, mP, w]),
                            prow.broadcast_to([P, 2, mP, w]),
                        )
                        nc.gpsimd.tensor_sub(
                            ga[:, b0:b1, r1:r1 + mP, k + 1:W],
                            ga[:, b0:b1, r1:r1 + mP, k + 1:W],
                            tbP[h][:, :, :mP, :w],
                        )

                # ---- backward substitution on the rhs column (Pool) ----
                for k in range(DP - 1, -1, -1):
                    nc.gpsimd.tensor_mul(
                        xsol[:, b0:b1, k], ga[:, b0:b1, k, DP],
                        invp[:, b0:b1, k],
                    )
                    if k == 0:
                        break
                    nc.gpsimd.tensor_mul(
                        bsc[h][:, :, :k],
                        ga[:, b0:b1, 0:k, k],
                        xsol[:, b0:b1, k:k + 1].broadcast_to([P, 2, k]),
                    )
                    nc.gpsimd.tensor_sub(
                        ga[:, b0:b1, 0:k, DP], ga[:, b0:b1, 0:k, DP],
                        bsc[h][:, :, :k],
                    )

                # ---- predictions: ypred = xtild_test . beta ----
                nc.gpsimd.tensor_mul(
                    prod[h], xtt[:, b0:b1], xsol[:, b0:b1])
                nc.vector.tensor_reduce(
                    yp[:, b0:b1], prod[h],
                    mybir.AxisListType.X, mybir.AluOpType.add,
                )

            nc.sync.dma_start(
                out_d.rearrange("(t p) one -> p (t one)", p=P), yp
            )

    nc.finalize()
    return nc


_cache: dict[float, object] = {}


def _get_nc(c2: float):
    if c2 not in _cache:
        _cache[c2] = _build_nc(c2)
    return _cache[c2]


def _build_xT(Xtrain, shard):
    """Pack [XtrT | XteT] with chunks at partition offsets 32g for 4-way
    row-group gram matmuls."""
    out = np.zeros((P, 4 * P + TS), np.float32)
    XtrT = Xtrain.T
    for g in range(4):
        for cc in range(4):
            c = 4 * cc + g
            out[32 * g:32 * g + D, cc * P:(cc + 1) * P] = \
                XtrT[:, c * P:(c + 1) * P]
        out[32 * g:32 * g + D, 4 * P:] = shard.T
    return out


def _host_pack(Ytrain, Xtrain, c2):
    """Train-side packing shared by all cores: Z expansion + exp biases."""
    Xt = np.concatenate(
        [np.ones((N_TRAIN, 1), np.float32), Xtrain], axis=1)  # [2048, 32]
    A = (Xt[:, :, None] * Xt[:, None, H:DP]).reshape(N_TRAIN, DP * H)
    B = (Xt[:, :H, None] * Xt[:, None, :H]).reshape(N_TRAIN, H * H)
    C = Xt * Ytrain[:, 0:1]
    zz = np.concatenate([A, B, C], axis=1)              # [2048, 800]
    zz = np.ascontiguousarray(
        zz.reshape(NK, P, NZ).transpose(1, 0, 2).reshape(P, NK * NZ))
    sn = np.sum(Xtrain * Xtrain, axis=1)                # [2048]
    bias_n = np.ascontiguousarray(
        (-0.5 * c2 * sn).reshape(NK, P).T.astype(np.float32))
    return zz, bias_n


def _host_pack_test(shard, c2):
    """Test-side packing per core: ridge scale + design rows."""
    st = np.sum(shard * shard, axis=1)                  # [512]
    regt = np.ascontiguousarray(
        (REG * np.exp(0.5 * c2 * st)).reshape(NT, P).T.astype(np.float32))
    xtt = np.concatenate(
        [np.ones((TS, 1), np.float32), shard], axis=1)  # [512, 32]
    xtt = np.ascontiguousarray(
        xtt.reshape(NT, P, DP).transpose(1, 0, 2).reshape(P, NT * DP))
    return regt, xtt


def kernel(Ytrain, Xtrain, Xtest, log_lengthscale, _trace=False):
    Ytrain = np.ascontiguousarray(np.asarray(Ytrain, dtype=np.float32))
    Xtrain = np.ascontiguousarray(np.asarray(Xtrain, dtype=np.float32))
    Xtest = np.ascontiguousarray(np.asarray(Xtest, dtype=np.float32))
    lls = float(np.asarray(log_lengthscale, dtype=np.float32))
    c2 = float(np.exp(np.float32(-2.0 * lls)))

    nc = _get_nc(c2)
    zz, bias_n = _host_pack(Ytrain, Xtrain, c2)
    in_maps = []
    for core in range(NCORES):
        shard = np.ascontiguousarray(Xtest[core * TS:(core + 1) * TS])
        regt, xtt = _host_pack_test(shard, c2)
        in_maps.append({
            "xT": _build_xT(Xtrain, shard),
            "zz": zz,
            "bias_n": bias_n,
            "regt": regt,
            "xtt": xtt,
        })
    res = run_bass_kernel_spmd(nc, in_maps, list(range(NCORES)),
                               trace=bool(_trace))
    outs = [np.asarray(res.results[c]["ypred"], dtype=np.float32)
            for c in range(NCORES)]
    full = np.concatenate(outs, axis=0)
    if _trace:
        return full, res
    return full


def _sim_in_map(inputs):
    """Core-0 input map for CoreSim timing (test.py helper)."""
    Ytrain = np.asarray(inputs["Ytrain"], dtype=np.float32)
    Xtrain = np.asarray(inputs["Xtrain"], dtype=np.float32)
    Xtest = np.asarray(inputs["Xtest"], dtype=np.float32)
    lls = float(np.asarray(inputs["log_lengthscale"], dtype=np.float32))
    c2 = float(np.exp(np.float32(-2.0 * lls)))
    shard = np.ascontiguousarray(Xtest[:TS])
    zz, bias_n = _host_pack(Ytrain, Xtrain, c2)
    regt, xtt = _host_pack_test(shard, c2)
    return c2, {
        "xT": _build_xT(Xtrain, shard),
        "zz": zz,
        "bias_n": bias_n,
        "regt": regt,
        "xtt": xtt,
    }


# revision 21
# speedup vs baseline: 1.2578x; 1.0574x over previous
"""Trainium2 Bass kernel: batched locally-weighted ridge regression.

Per test point t: K[t,n] = exp(-|xte_t - xtr_n|^2 / (2 ls^2));
  A_t = Xtild^T diag(K[t]) Xtild + REG*I ; b_t = Xtild^T (K[t] * Y)
  ypred_t = xtild_t . A_t^{-1} b_t
Sharding: data-parallel over the 4096 test points -> 8 cores x 512.

On-device math uses a scaled kernel K'[t,n] = exp((S[n,t] - sn[n]/2) * c2)
(c2 = 1/ls^2), i.e. the exp(-st*c2/2) per-test factor is dropped; this
rescales A_t and b_t identically, so beta is preserved by using a
per-test ridge REG_t = REG * exp(st*c2/2).

v2 layout:
  - Host precomputes the outer-product expansion Z = [x_d*x_e | x*y]
    (train-only data), squared-norm exp biases, per-test ridge, and the
    test design rows; DMA'd in parallel streams.
  - PE: warmup matmuls (clock ramp), gram S = XtrT-groups @ XteT in
    f32r, then XWX/XWy accumulation K'-chunks @ Z in f32r.
  - ACT: exp(S*c2 + bias), PSUM evacuations into [A|b] systems, mirror.
  - DVE+Pool: batched Gaussian elimination, 2 halves of 2x128 systems,
    rows of each update split DVE (top) / Pool (bottom); back-subst
    and prediction dot products on Pool; reductions/reciprocals on DVE.
"""

import numpy as np

import concourse.bacc as bacc
import concourse.mybir as mybir
from concourse.bass import ds, ts
from concourse.bass_utils import run_bass_kernel_spmd
from concourse.tile import TileContext

F32 = mybir.dt.float32
F32R = mybir.dt.float32r
P = 128
N_TRAIN = 2048
D = 31
DP = 32          # 1 + D
W = 33           # DP + rhs column
N_TEST = 4096
NCORES = 8
TS = N_TEST // NCORES   # 512 test points per core
NT = TS // P            # 4 t-tiles
NK = N_TRAIN // P       # 16 train chunks
NZ = 800                # 512 (d x e>=16) + 256 (d<16 x e<16) + 32 (x*y)
REG = 1e-6
H = 16

N_WARMUP = 4            # PE clock-ramp warmup matmuls (fp32, 4-pass)
POOL_FRAC = 0.63        # fraction of elimination rows on the Pool engine


def _pool_rows(m: int) -> int:
    if m <= 3:
        return 0
    return min(m - 1, int(m * POOL_FRAC + 0.5))


def _build_nc(c2: float):
    """Build the single-core Bass program (SPMD across 8 cores)."""
    nc = bacc.Bacc(trn_type="TRN2")

    # transposed features packed for 4-way row-group gram matmuls:
    # [32g+d, cc*128+p] = Xtrain[(4*cc+g)*128+p, d]; cols 512: = XtestT x4
    xT_d = nc.dram_tensor("xT", [P, 4 * P + TS], F32R, kind="ExternalInput")
    zz_d = nc.dram_tensor("zz", [P, NK * NZ], F32R, kind="ExternalInput")
    regt_d = nc.dram_tensor("regt", [P, NT], F32, kind="ExternalInput")
    xtt_d = nc.dram_tensor("xtt", [P, NT * DP], F32, kind="ExternalInput")
    out_d = nc.dram_tensor("ypred", [TS, 1], F32, kind="ExternalOutput")

    with TileContext(nc) as tc:
        with (
            tc.tile_pool(name="sb", bufs=1) as sb,
            tc.tile_pool(name="pgram", bufs=2, space="PSUM") as pgram,
            tc.tile_pool(name="pxwx", bufs=4, space="PSUM") as pxwx,
        ):
            # ---- input loads; zz spread across engine DMA queues so the
            # transfers run in parallel (issuing engine is charged the
            # transfer time in the DGE model)
            xT = sb.tile([P, 4 * P + TS], F32R)
            nc.sync.dma_start(xT[:, 0:512], xT_d[:, 0:512])
            nc.sync.dma_start(xT[:, 512:], xT_d[:, 512:])

            zz = sb.tile([P, NK, NZ], F32R)
            zr = zz[:].rearrange("p c z -> p (c z)")
            zq = [nc.sync, nc.gpsimd, nc.sync, nc.gpsimd,
                  nc.sync, nc.gpsimd, nc.sync, nc.gpsimd]
            for q in range(8):
                zq[q].dma_start(
                    zr[:, ts(q, 2 * NZ)], zz_d[:, ts(q, 2 * NZ)]
                )
            regt = sb.tile([P, NT], F32)
            nc.sync.dma_start(regt, regt_d[:, :])
            xtt = sb.tile([P, NT, DP], F32)
            nc.sync.dma_start(
                xtt, xtt_d.rearrange("p (t d) -> p t d", t=NT)
            )

            # ---- PE warmup: ramp the clock during the DMAs ----
            wu = sb.tile([P, 256], F32)
            nc.vector.memset(wu, 1.0)
            for _ in range(N_WARMUP):
                wps = pxwx.tile([P, 256], F32, tag="px")
                nc.tensor.matmul(wps, wu[0:8, 0:128], wu[0:8, :],
                                 start=True, stop=True)

            # ---- gram + K' = exp((S - sn/2)*c2), layout [n_chunk, t];
            # the -sn/2 bias rides in row 31 of each xT group (x the ones
            # row on the test side), so exps batch over chunk pairs ----
            kp = sb.tile([P, NK, TS], F32R)
            for cp in range(NK // 2):
                sg = pgram.tile([P, 2, TS], F32, tag="sg")
                for i in range(2):
                    c = 2 * cp + i
                    cc, g = c // 4, c % 4
                    nc.tensor.matmul(
                        sg[:, i, :],
                        xT[32 * g:32 * g + DP, ts(cc, P)],
                        xT[32 * g:32 * g + DP, 4 * P:],
                        start=True, stop=True,
                        tile_position=(32 * g, 0),
                    )
                nc.scalar.activation(
                    kp[:, 2 * cp:2 * cp + 2, :], sg,
                    mybir.ActivationFunctionType.Exp,
                    scale=c2,
                )

            # ---- per-half: XWX/XWy matmuls, assembly, solve, predict ----
            ga = sb.tile([P, NT, DP, W], F32)
            invp = sb.tile([P, NT, DP], F32)
            xsol = sb.tile([P, NT, DP], F32)
            yp = sb.tile([P, NT], F32)
            fbD = [sb.tile([P, 2, D], F32, name=f"fbD{i}")
                   for i in range(2)]
            tbD = [sb.tile([P, 2, D, DP], F32, name=f"tbD{i}")
                   for i in range(2)]
            fbP = [sb.tile([P, 2, D], F32, name=f"fbP{i}")
                   for i in range(2)]
            tbP = [sb.tile([P, 2, D, DP], F32, name=f"tbP{i}")
                   for i in range(2)]
            bsc = [sb.tile([P, 2, D], F32, name=f"bsc{i}")
                   for i in range(2)]
            prod = [sb.tile([P, 2, DP], F32, name=f"prod{i}")
                   for i in range(2)]
            ga_sw = ga[:].rearrange("p b r c -> p b c r")
            ga_diag = ga[:].rearrange("p b r c -> p b (r c)")[:, :, ::W + 1]

            for h in range(2):
                b0, b1 = 2 * h, 2 * h + 2
                # XWX | XWy: [256, 1056] per half via K'-chunks @ Z
                for t in range(b0, b1):
                    for (c0, c1) in ((0, 512), (512, NZ)):
                        w = c1 - c0
                        px = pxwx.tile([P, 512], F32, tag="px")
                        for c in range(NK):
                            nc.tensor.matmul(
                                px[:, :w],
                                kp[:, c, ts(t, P)],
                                zz[:, c, c0:c1],
                                start=(c == 0), stop=(c == NK - 1),
                            )
                        if c0 == 0:
                            # cols e=16..31, all rows d
                            nc.scalar.copy(
                                ga[:, t, :, H:DP],
                                px[:, :w].rearrange("p (r c) -> p r c", r=DP),
                            )
                        else:
                            # top-left quadrant + rhs column
                            nc.scalar.copy(
                                ga[:, t, 0:H, 0:H],
                                px[:, 0:H * H].rearrange(
                                    "p (r c) -> p r c", r=H),
                            )
                            nc.scalar.copy(
                                ga[:, t, :, DP], px[:, H * H:H * H + DP])

                # mirror lower-left quadrant from upper-right (A symmetric)
                nc.scalar.copy(
                    ga[:, b0:b1, H:DP, 0:H],
                    ga_sw[:, b0:b1, H:DP, 0:H],
                )
                # per-test ridge on the diagonal
                nc.gpsimd.tensor_add(
                    ga_diag[:, b0:b1], ga_diag[:, b0:b1],
                    regt[:, b0:b1, None].broadcast_to([P, 2, DP]),
                )

                # ---- batched Gaussian elimination (no pivoting; A SPD),
                # update rows split: DVE takes the top mD, Pool bottom mP
                for k in range(DP):
                    nc.vector.reciprocal(
                        invp[:, b0:b1, k], ga[:, b0:b1, k, k])
                    if k == DP - 1:
                        break
                    m = D - k
                    w = DP - k
                    mP = _pool_rows(m)
                    mD = m - mP
                    prow = ga[:, b0:b1, k:k + 1, k + 1:W]
                    iv = invp[:, b0:b1, k:k + 1]
                    r0 = k + 1
                    nc.vector.tensor_mul(
                        fbD[h][:, :, :mD],
                        ga[:, b0:b1, r0:r0 + mD, k],
                        iv.broadcast_to([P, 2, mD]),
                    )
                    nc.vector.tensor_mul(
                        tbD[h][:, :, :mD, :w],
                        fbD[h][:, :, :mD, None].broadcast_to([P, 2, mD, w]),
                        prow.broadcast_to([P, 2, mD, w]),
                    )
                    nc.vector.tensor_sub(
                        ga[:, b0:b1, r0:r0 + mD, k + 1:W],
                        ga[:, b0:b1, r0:r0 + mD, k + 1:W],
                        tbD[h][:, :, :mD, :w],
                    )
                    if mP:
                        r1 = r0 + mD
                        nc.gpsimd.tensor_mul(
                            fbP[h][:, :, :mP],
                            ga[:, b0:b1, r1:r1 + mP, k],
                            iv.broadcast_to([P, 2, mP]),
                        )
                        nc.gpsimd.tensor_mul(
                            tbP[h][:, :, :mP, :w],
                            fbP[h][:, :, :mP, None].broadcast_to(
                                [P, 2, mP, w]),
                            prow.broadcast_to([P, 2, mP, w]),
                        )
                        nc.gpsimd.tensor_sub(
                            ga[:, b0:b1, r1:r1 + mP, k + 1:W],
                            ga[:, b0:b1, r1:r1 + mP, k + 1:W],
                            tbP[h][:, :, :mP, :w],
                        )

                # ---- backward substitution on the rhs column (Pool) ----
                for k in range(DP - 1, -1, -1):
                    nc.gpsimd.tensor_mul(
                        xsol[:, b0:b1, k], ga[:, b0:b1, k, DP],
                        invp[:, b0:b1, k],
                    )
                    if k == 0:
                        break
                    nc.gpsimd.tensor_mul(
                        bsc[h][:, :, :k],
                        ga[:, b0:b1, 0:k, k],
                        xsol[:, b0:b1, k:k + 1].broadcast_to([P, 2, k]),
                    )
                    nc.gpsimd.tensor_sub(
                        ga[:, b0:b1, 0:k, DP], ga[:, b0:b1, 0:k, DP],
                        bsc[h][:, :, :k],
                    )

                # ---- predictions: ypred = xtild_test . beta ----
                nc.gpsimd.tensor_mul(
                    prod[h], xtt[:, b0:b1], xsol[:, b0:b1])
                nc.vector.tensor_reduce(
                    yp[:, b0:b1], prod[h],
                    mybir.AxisListType.X, mybir.AluOpType.add,
                )

            nc.sync.dma_start(
                out_d.rearrange("(t p) one -> p (t one)", p=P), yp
            )

    nc.finalize()
    return nc


_cache: dict[float, object] = {}


def _get_nc(c2: float):
    if c2 not in _cache:
        _cache[c2] = _build_nc(c2)
    return _cache[c2]


def _build_xT(Xtrain, shard):
    """Pack [XtrT | XteT] with chunks at partition offsets 32g for 4-way
    row-group gram matmuls.  Row 31 of each group carries -|x|^2/2 on the
    train side and 1.0 on the test side, so the gram matmul computes
    S - sn/2 directly (no separate exp bias)."""
    out = np.zeros((P, 4 * P + TS), np.float32)
    XtrT = Xtrain.T
    nsn2 = -0.5 * np.sum(Xtrain * Xtrain, axis=1)       # [2048]
    for g in range(4):
        for cc in range(4):
            c = 4 * cc + g
            out[32 * g:32 * g + D, cc * P:(cc + 1) * P] = \
                XtrT[:, c * P:(c + 1) * P]
            out[32 * g + D, cc * P:(cc + 1) * P] = nsn2[c * P:(c + 1) * P]
        out[32 * g:32 * g + D, 4 * P:] = shard.T
        out[32 * g + D, 4 * P:] = 1.0
    return out


def _host_pack(Ytrain, Xtrain):
    """Train-side packing shared by all cores: the Z expansion."""
    Xt = np.concatenate(
        [np.ones((N_TRAIN, 1), np.float32), Xtrain], axis=1)  # [2048, 32]
    A = (Xt[:, :, None] * Xt[:, None, H:DP]).reshape(N_TRAIN, DP * H)
    B = (Xt[:, :H, None] * Xt[:, None, :H]).reshape(N_TRAIN, H * H)
    C = Xt * Ytrain[:, 0:1]
    zz = np.concatenate([A, B, C], axis=1)              # [2048, 800]
    return np.ascontiguousarray(
        zz.reshape(NK, P, NZ).transpose(1, 0, 2).reshape(P, NK * NZ))


def _host_pack_test(shard, c2):
    """Test-side packing per core: ridge scale + design rows."""
    st = np.sum(shard * shard, axis=1)                  # [512]
    regt = np.ascontiguousarray(
        (REG * np.exp(0.5 * c2 * st)).reshape(NT, P).T.astype(np.float32))
    xtt = np.concatenate(
        [np.ones((TS, 1), np.float32), shard], axis=1)  # [512, 32]
    xtt = np.ascontiguousarray(
        xtt.reshape(NT, P, DP).transpose(1, 0, 2).reshape(P, NT * DP))
    return regt, xtt


def kernel(Ytrain, Xtrain, Xtest, log_lengthscale, _trace=False):
    Ytrain = np.ascontiguousarray(np.asarray(Ytrain, dtype=np.float32))
    Xtrain = np.ascontiguousarray(np.asarray(Xtrain, dtype=np.float32))
    Xtest = np.ascontiguousarray(np.asarray(Xtest, dtype=np.float32))
    lls = float(np.asarray(log_lengthscale, dtype=np.float32))
    c2 = float(np.exp(np.float32(-2.0 * lls)))

    nc = _get_nc(c2)
    zz = _host_pack(Ytrain, Xtrain)
    in_maps = []
    for core in range(NCORES):
        shard = np.ascontiguousarray(Xtest[core * TS:(core + 1) * TS])
        regt, xtt = _host_pack_test(shard, c2)
        in_maps.append({
            "xT": _build_xT(Xtrain, shard),
            "zz": zz,
            "regt": regt,
            "xtt": xtt,
        })
    res = run_bass_kernel_spmd(nc, in_maps, list(range(NCORES)),
                               trace=bool(_trace))
    outs = [np.asarray(res.results[c]["ypred"], dtype=np.float32)
            for c in range(NCORES)]
    full = np.concatenate(outs, axis=0)
    if _trace:
        return full, res
    return full


def _sim_in_map(inputs):
    """Core-0 input map for CoreSim timing (test.py helper)."""
    Ytrain = np.asarray(inputs["Ytrain"], dtype=np.float32)
    Xtrain = np.asarray(inputs["Xtrain"], dtype=np.float32)
    Xtest = np.asarray(inputs["Xtest"], dtype=np.float32)
    lls = float(np.asarray(inputs["log_lengthscale"], dtype=np.float32))
    c2 = float(np.exp(np.float32(-2.0 * lls)))
    shard = np.ascontiguousarray(Xtest[:TS])
    zz = _host_pack(Ytrain, Xtrain)
    regt, xtt = _host_pack_test(shard, c2)
    return c2, {
        "xT": _build_xT(Xtrain, shard),
        "zz": zz,
        "regt": regt,
        "xtt": xtt,
    }


# revision 25
# speedup vs baseline: 1.3746x; 1.0928x over previous
"""Trainium2 Bass kernel: batched locally-weighted ridge regression.

Per test point t: K[t,n] = exp(-|xte_t - xtr_n|^2 / (2 ls^2));
  A_t = Xtild^T diag(K[t]) Xtild + REG*I ; b_t = Xtild^T (K[t] * Y)
  ypred_t = xtild_t . A_t^{-1} b_t
Sharding: data-parallel over the 4096 test points -> 8 cores x 512.

On-device math uses a scaled kernel K'[t,n] = exp((S[n,t] - sn[n]/2) * c2)
(c2 = 1/ls^2), i.e. the exp(-st*c2/2) per-test factor is dropped; this
rescales A_t and b_t identically, so beta is preserved by using a
per-test ridge REG_t = REG * exp(st*c2/2).

v2 layout:
  - Host precomputes the outer-product expansion Z = [x_d*x_e | x*y]
    (train-only data), squared-norm exp biases, per-test ridge, and the
    test design rows; DMA'd in parallel streams.
  - PE: warmup matmuls (clock ramp), gram S = XtrT-groups @ XteT in
    f32r, then XWX/XWy accumulation K'-chunks @ Z in f32r.
  - ACT: exp(S*c2 + bias), PSUM evacuations into [A|b] systems, mirror.
  - DVE+Pool: batched Gaussian elimination, 2 halves of 2x128 systems,
    rows of each update split DVE (top) / Pool (bottom); back-subst
    and prediction dot products on Pool; reductions/reciprocals on DVE.
"""

import numpy as np

import concourse.bacc as bacc
import concourse.mybir as mybir
from concourse.bass import ds, ts
from concourse.bass_utils import run_bass_kernel_spmd
from concourse.tile import TileContext

F32 = mybir.dt.float32
F32R = mybir.dt.float32r
P = 128
N_TRAIN = 2048
D = 31
DP = 32          # 1 + D
W = 33           # DP + rhs column
N_TEST = 4096
NCORES = 8
TS = N_TEST // NCORES   # 512 test points per core
NT = TS // P            # 4 t-tiles
NK = N_TRAIN // P       # 16 train chunks
NZ = 800                # 512 (d x e>=16) + 256 (d<16 x e<16) + 32 (x*y)
REG = 1e-6
H = 16

N_WARMUP = 3            # PE clock-ramp warmup matmuls (fp32, 4-pass)
BAND = 4                # DVE pivot-band height (refill period)


def _build_nc(c2: float):
    """Build the single-core Bass program (SPMD across 8 cores)."""
    nc = bacc.Bacc(trn_type="TRN2")

    # transposed features packed for 4-way row-group gram matmuls:
    # [32g+d, cc*128+p] = Xtrain[(4*cc+g)*128+p, d]; cols 512: = XtestT x4
    xT_d = nc.dram_tensor("xT", [P, 4 * P + TS], F32R, kind="ExternalInput")
    zz_d = nc.dram_tensor("zz", [P, NK * NZ], F32R, kind="ExternalInput")
    regt_d = nc.dram_tensor("regt", [P, NT], F32, kind="ExternalInput")
    xtt_d = nc.dram_tensor("xtt", [P, NT * DP], F32, kind="ExternalInput")
    out_d = nc.dram_tensor("ypred", [TS, 1], F32, kind="ExternalOutput")

    with TileContext(nc) as tc:
        with (
            tc.tile_pool(name="sb", bufs=1) as sb,
            tc.tile_pool(name="pgram", bufs=2, space="PSUM") as pgram,
            tc.tile_pool(name="pxwx", bufs=4, space="PSUM") as pxwx,
        ):
            # ---- input loads; zz spread across engine DMA queues so the
            # transfers run in parallel (issuing engine is charged the
            # transfer time in the DGE model)
            xT = sb.tile([P, 4 * P + TS], F32R)
            nc.sync.dma_start(xT[:, 0:512], xT_d[:, 0:512])
            nc.sync.dma_start(xT[:, 512:], xT_d[:, 512:])

            zz = sb.tile([P, NK, NZ], F32R)
            zr = zz[:].rearrange("p c z -> p (c z)")
            zq = [nc.sync, nc.gpsimd, nc.sync, nc.gpsimd,
                  nc.sync, nc.gpsimd, nc.sync, nc.gpsimd]
            for q in range(8):
                zq[q].dma_start(
                    zr[:, ts(q, 2 * NZ)], zz_d[:, ts(q, 2 * NZ)]
                )
            regt = sb.tile([P, NT], F32)
            nc.sync.dma_start(regt, regt_d[:, :])
            xtt = sb.tile([P, NT, DP], F32)
            nc.sync.dma_start(
                xtt, xtt_d.rearrange("p (t d) -> p t d", t=NT)
            )

            # ---- PE warmup: ramp the clock during the DMAs ----
            wu = sb.tile([P, 256], F32)
            nc.vector.memset(wu, 1.0)
            for _ in range(N_WARMUP):
                wps = pxwx.tile([P, 256], F32, tag="px")
                nc.tensor.matmul(wps, wu[0:8, 0:128], wu[0:8, :],
                                 start=True, stop=True)

            # ---- gram + K' = exp((S - sn/2)*c2), layout [n_chunk, t];
            # the -sn/2 bias rides in row 31 of each xT group (x the ones
            # row on the test side), so exps batch over chunk pairs ----
            kp = sb.tile([P, NK, TS], F32R)
            for cp in range(NK // 2):
                sg = pgram.tile([P, 2, TS], F32, tag="sg")
                for i in range(2):
                    c = 2 * cp + i
                    cc, g = c // 4, c % 4
                    nc.tensor.matmul(
                        sg[:, i, :],
                        xT[32 * g:32 * g + DP, ts(cc, P)],
                        xT[32 * g:32 * g + DP, 4 * P:],
                        start=True, stop=True,
                        tile_position=(32 * g, 0),
                    )
                nc.scalar.activation(
                    kp[:, 2 * cp:2 * cp + 2, :], sg,
                    mybir.ActivationFunctionType.Exp,
                    scale=c2,
                )

            # ---- per-half: XWX/XWy matmuls, assembly, solve, predict ----
            ga = sb.tile([P, NT, DP, W], F32)
            invp = sb.tile([P, NT, DP], F32)
            xsol = sb.tile([P, NT, DP], F32)
            yp = sb.tile([P, NT], F32)
            psr = [sb.tile([P, 2, DP], F32, name=f"psr{i}")
                   for i in range(2)]
            tbD = [sb.tile([P, 2, BAND, DP], F32, name=f"tbD{i}")
                   for i in range(2)]
            tbP = [sb.tile([P, 2, DP, 16], F32, name=f"tbP{i}")
                   for i in range(2)]
            bsc = [sb.tile([P, 2, D], F32, name=f"bsc{i}")
                   for i in range(2)]
            prod = [sb.tile([P, 2, DP], F32, name=f"prod{i}")
                   for i in range(2)]
            ga_diag = ga[:].rearrange("p b r c -> p b (r c)")[:, :, ::W + 1]

            for h in range(2):
                b0, b1 = 2 * h, 2 * h + 2
                # XWX | XWy: [256, 1056] per half via K'-chunks @ Z
                for t in range(b0, b1):
                    for (c0, c1) in ((0, 512), (512, NZ)):
                        w = c1 - c0
                        px = pxwx.tile([P, 512], F32, tag="px")
                        for c in range(NK):
                            nc.tensor.matmul(
                                px[:, :w],
                                kp[:, c, ts(t, P)],
                                zz[:, c, c0:c1],
                                start=(c == 0), stop=(c == NK - 1),
                            )
                        if c0 == 0:
                            # cols e=16..31, all rows d
                            nc.scalar.copy(
                                ga[:, t, :, H:DP],
                                px[:, :w].rearrange("p (r c) -> p r c", r=DP),
                            )
                        else:
                            # top-left quadrant + rhs column
                            nc.scalar.copy(
                                ga[:, t, 0:H, 0:H],
                                px[:, 0:H * H].rearrange(
                                    "p (r c) -> p r c", r=H),
                            )
                            nc.scalar.copy(
                                ga[:, t, :, DP], px[:, H * H:H * H + DP])

                # per-test ridge on the diagonal
                nc.gpsimd.tensor_add(
                    ga_diag[:, b0:b1], ga_diag[:, b0:b1],
                    regt[:, b0:b1, None].broadcast_to([P, 2, DP]),
                )
                # lower-left quadrant is never written by the evacuations
                # (symmetric solve reads upper only) but band rects read
                # across it; zero it so values stay finite
                nc.gpsimd.memset(ga[:, b0:b1, H:DP, 0:H], 0.0)

                # ---- symmetric (LDLt-style) elimination: A[i,k] = A[k,i],
                # so the update is upd[i,j] = ps[i]*prow[j] with ps = prow
                # *invp, and only the upper triangle + rhs is ever read.
                # DVE keeps a BAND-row window at the pivot (refilled every
                # BAND steps); Pool updates the remaining upper triangle as
                # column-panel rectangles.
                for k in range(DP):
                    nc.vector.reciprocal(
                        invp[:, b0:b1, k], ga[:, b0:b1, k, k])
                    if k == DP - 1:
                        break
                    m = D - k                  # rows k+1..31
                    w = W - 1 - k              # cols k+1..32 (incl rhs)
                    mD = min(BAND - (k % BAND), m)
                    r1 = k + 1 + mD            # first Pool row
                    prow = ga[:, b0:b1, k:k + 1, k + 1:W]
                    ps = psr[h]
                    nc.vector.tensor_mul(
                        ps[:, :, :w], ga[:, b0:b1, k, k + 1:W],
                        invp[:, b0:b1, k:k + 1].broadcast_to([P, 2, w]),
                    )
                    # DVE band: rows k+1..r1, all cols k+1..33
                    nc.vector.tensor_mul(
                        tbD[h][:, :, :mD, :w],
                        ps[:, :, :mD, None].broadcast_to([P, 2, mD, w]),
                        prow.broadcast_to([P, 2, mD, w]),
                    )
                    nc.vector.tensor_sub(
                        ga[:, b0:b1, k + 1:r1, k + 1:W],
                        ga[:, b0:b1, k + 1:r1, k + 1:W],
                        tbD[h][:, :, :mD, :w],
                    )
                    # Pool: upper-triangle rows r1..31 by column panels
                    for p in range(r1 // 8, 4):
                        re_ = min(8 * p + 8, DP)
                        if re_ <= r1:
                            continue
                        cs = max(8 * p, k + 1)
                        ce = W if p == 3 else 8 * p + 8
                        mr, wc = re_ - r1, ce - cs
                        nc.gpsimd.tensor_mul(
                            tbP[h][:, :, :mr, :wc],
                            ps[:, :, r1 - k - 1:re_ - k - 1, None]
                            .broadcast_to([P, 2, mr, wc]),
                            ga[:, b0:b1, k:k + 1, cs:ce]
                            .broadcast_to([P, 2, mr, wc]),
                        )
                        nc.gpsimd.tensor_sub(
                            ga[:, b0:b1, r1:re_, cs:ce],
                            ga[:, b0:b1, r1:re_, cs:ce],
                            tbP[h][:, :, :mr, :wc],
                        )

                # ---- backward substitution on the rhs column (Pool) ----
                for k in range(DP - 1, -1, -1):
                    nc.gpsimd.tensor_mul(
                        xsol[:, b0:b1, k], ga[:, b0:b1, k, DP],
                        invp[:, b0:b1, k],
                    )
                    if k == 0:
                        break
                    nc.gpsimd.tensor_mul(
                        bsc[h][:, :, :k],
                        ga[:, b0:b1, 0:k, k],
                        xsol[:, b0:b1, k:k + 1].broadcast_to([P, 2, k]),
                    )
                    nc.gpsimd.tensor_sub(
                        ga[:, b0:b1, 0:k, DP], ga[:, b0:b1, 0:k, DP],
                        bsc[h][:, :, :k],
                    )

                # ---- predictions: ypred = xtild_test . beta ----
                nc.gpsimd.tensor_mul(
                    prod[h], xtt[:, b0:b1], xsol[:, b0:b1])
                nc.vector.tensor_reduce(
                    yp[:, b0:b1], prod[h],
                    mybir.AxisListType.X, mybir.AluOpType.add,
                )

            nc.sync.dma_start(
                out_d.rearrange("(t p) one -> p (t one)", p=P), yp
            )

    nc.finalize()
    return nc


_cache: dict[float, object] = {}


def _get_nc(c2: float):
    if c2 not in _cache:
        _cache[c2] = _build_nc(c2)
    return _cache[c2]


def _build_xT(Xtrain, shard):
    """Pack [XtrT | XteT] with chunks at partition offsets 32g for 4-way
    row-group gram matmuls.  Row 31 of each group carries -|x|^2/2 on the
    train side and 1.0 on the test side, so the gram matmul computes
    S - sn/2 directly (no separate exp bias)."""
    out = np.zeros((P, 4 * P + TS), np.float32)
    XtrT = Xtrain.T
    nsn2 = -0.5 * np.sum(Xtrain * Xtrain, axis=1)       # [2048]
    for g in range(4):
        for cc in range(4):
            c = 4 * cc + g
            out[32 * g:32 * g + D, cc * P:(cc + 1) * P] = \
                XtrT[:, c * P:(c + 1) * P]
            out[32 * g + D, cc * P:(cc + 1) * P] = nsn2[c * P:(c + 1) * P]
        out[32 * g:32 * g + D, 4 * P:] = shard.T
        out[32 * g + D, 4 * P:] = 1.0
    return out


def _host_pack(Ytrain, Xtrain):
    """Train-side packing shared by all cores: the Z expansion."""
    Xt = np.concatenate(
        [np.ones((N_TRAIN, 1), np.float32), Xtrain], axis=1)  # [2048, 32]
    A = (Xt[:, :, None] * Xt[:, None, H:DP]).reshape(N_TRAIN, DP * H)
    B = (Xt[:, :H, None] * Xt[:, None, :H]).reshape(N_TRAIN, H * H)
    C = Xt * Ytrain[:, 0:1]
    zz = np.concatenate([A, B, C], axis=1)              # [2048, 800]
    return np.ascontiguousarray(
        zz.reshape(NK, P, NZ).transpose(1, 0, 2).reshape(P, NK * NZ))


def _host_pack_test(shard, c2):
    """Test-side packing per core: ridge scale + design rows."""
    st = np.sum(shard * shard, axis=1)                  # [512]
    regt = np.ascontiguousarray(
        (REG * np.exp(0.5 * c2 * st)).reshape(NT, P).T.astype(np.float32))
    xtt = np.concatenate(
        [np.ones((TS, 1), np.float32), shard], axis=1)  # [512, 32]
    xtt = np.ascontiguousarray(
        xtt.reshape(NT, P, DP).transpose(1, 0, 2).reshape(P, NT * DP))
    return regt, xtt


def kernel(Ytrain, Xtrain, Xtest, log_lengthscale, _trace=False):
    Ytrain = np.ascontiguousarray(np.asarray(Ytrain, dtype=np.float32))
    Xtrain = np.ascontiguousarray(np.asarray(Xtrain, dtype=np.float32))
    Xtest = np.ascontiguousarray(np.asarray(Xtest, dtype=np.float32))
    lls = float(np.asarray(log_lengthscale, dtype=np.float32))
    c2 = float(np.exp(np.float32(-2.0 * lls)))

    nc = _get_nc(c2)
    zz = _host_pack(Ytrain, Xtrain)
    in_maps = []
    for core in range(NCORES):
        shard = np.ascontiguousarray(Xtest[core * TS:(core + 1) * TS])
        regt, xtt = _host_pack_test(shard, c2)
        in_maps.append({
            "xT": _build_xT(Xtrain, shard),
            "zz": zz,
            "regt": regt,
            "xtt": xtt,
        })
    res = run_bass_kernel_spmd(nc, in_maps, list(range(NCORES)),
                               trace=bool(_trace))
    outs = [np.asarray(res.results[c]["ypred"], dtype=np.float32)
            for c in range(NCORES)]
    full = np.concatenate(outs, axis=0)
    if _trace:
        return full, res
    return full


def _sim_in_map(inputs):
    """Core-0 input map for CoreSim timing (test.py helper)."""
    Ytrain = np.asarray(inputs["Ytrain"], dtype=np.float32)
    Xtrain = np.asarray(inputs["Xtrain"], dtype=np.float32)
    Xtest = np.asarray(inputs["Xtest"], dtype=np.float32)
    lls = float(np.asarray(inputs["log_lengthscale"], dtype=np.float32))
    c2 = float(np.exp(np.float32(-2.0 * lls)))
    shard = np.ascontiguousarray(Xtest[:TS])
    zz = _host_pack(Ytrain, Xtrain)
    regt, xtt = _host_pack_test(shard, c2)
    return c2, {
        "xT": _build_xT(Xtrain, shard),
        "zz": zz,
        "regt": regt,
        "xtt": xtt,
    }


# revision 26
# speedup vs baseline: 1.4077x; 1.0241x over previous
"""Trainium2 Bass kernel: batched locally-weighted ridge regression.

Per test point t: K[t,n] = exp(-|xte_t - xtr_n|^2 / (2 ls^2));
  A_t = Xtild^T diag(K[t]) Xtild + REG*I ; b_t = Xtild^T (K[t] * Y)
  ypred_t = xtild_t . A_t^{-1} b_t
Sharding: data-parallel over the 4096 test points -> 8 cores x 512.

On-device math uses a scaled kernel K'[t,n] = exp((S[n,t] - sn[n]/2) * c2)
(c2 = 1/ls^2), i.e. the exp(-st*c2/2) per-test factor is dropped; this
rescales A_t and b_t identically, so beta is preserved by using a
per-test ridge REG_t = REG * exp(st*c2/2).

v2 layout:
  - Host precomputes the outer-product expansion Z = [x_d*x_e | x*y]
    (train-only data), squared-norm exp biases, per-test ridge, and the
    test design rows; DMA'd in parallel streams.
  - PE: warmup matmuls (clock ramp), gram S = XtrT-groups @ XteT in
    f32r, then XWX/XWy accumulation K'-chunks @ Z in f32r.
  - ACT: exp(S*c2 + bias), PSUM evacuations into [A|b] systems, mirror.
  - DVE+Pool: batched Gaussian elimination, 2 halves of 2x128 systems,
    rows of each update split DVE (top) / Pool (bottom); back-subst
    and prediction dot products on Pool; reductions/reciprocals on DVE.
"""

import numpy as np

import concourse.bacc as bacc
import concourse.mybir as mybir
from concourse.bass import ds, ts
from concourse.bass_utils import run_bass_kernel_spmd
from concourse.tile import TileContext

F32 = mybir.dt.float32
F32R = mybir.dt.float32r
P = 128
N_TRAIN = 2048
D = 31
DP = 32          # 1 + D
W = 33           # DP + rhs column
N_TEST = 4096
NCORES = 8
TS = N_TEST // NCORES   # 512 test points per core
NT = TS // P            # 4 t-tiles
NK = N_TRAIN // P       # 16 train chunks
NZ = 800                # 512 (d x e>=16) + 256 (d<16 x e<16) + 32 (x*y)
REG = 1e-6
H = 16

N_WARMUP = 3            # PE clock-ramp warmup matmuls (fp32, 4-pass)
BAND = 8                # DVE pivot-band height (refill period)


def _build_nc(c2: float):
    """Build the single-core Bass program (SPMD across 8 cores)."""
    nc = bacc.Bacc(trn_type="TRN2")

    # transposed features packed for 4-way row-group gram matmuls:
    # [32g+d, cc*128+p] = Xtrain[(4*cc+g)*128+p, d]; cols 512: = XtestT x4
    xT_d = nc.dram_tensor("xT", [P, 4 * P + TS], F32R, kind="ExternalInput")
    zz_d = nc.dram_tensor("zz", [P, NK * NZ], F32R, kind="ExternalInput")
    regt_d = nc.dram_tensor("regt", [P, NT], F32, kind="ExternalInput")
    xtt_d = nc.dram_tensor("xtt", [P, NT * DP], F32, kind="ExternalInput")
    out_d = nc.dram_tensor("ypred", [TS, 1], F32, kind="ExternalOutput")

    with TileContext(nc) as tc:
        with (
            tc.tile_pool(name="sb", bufs=1) as sb,
            tc.tile_pool(name="pgram", bufs=2, space="PSUM") as pgram,
            tc.tile_pool(name="pxwx", bufs=4, space="PSUM") as pxwx,
        ):
            # ---- input loads; zz spread across engine DMA queues so the
            # transfers run in parallel (issuing engine is charged the
            # transfer time in the DGE model)
            xT = sb.tile([P, 4 * P + TS], F32R)
            nc.sync.dma_start(xT[:, 0:512], xT_d[:, 0:512])
            nc.sync.dma_start(xT[:, 512:], xT_d[:, 512:])

            zz = sb.tile([P, NK, NZ], F32R)
            zr = zz[:].rearrange("p c z -> p (c z)")
            zq = [nc.sync, nc.gpsimd, nc.sync, nc.gpsimd,
                  nc.sync, nc.gpsimd, nc.sync, nc.gpsimd]
            for q in range(8):
                zq[q].dma_start(
                    zr[:, ts(q, 2 * NZ)], zz_d[:, ts(q, 2 * NZ)]
                )
            regt = sb.tile([P, NT], F32)
            nc.sync.dma_start(regt, regt_d[:, :])
            xtt = sb.tile([P, NT, DP], F32)
            nc.sync.dma_start(
                xtt, xtt_d.rearrange("p (t d) -> p t d", t=NT)
            )

            # ---- PE warmup: ramp the clock during the DMAs ----
            wu = sb.tile([P, 256], F32)
            nc.vector.memset(wu, 1.0)
            for _ in range(N_WARMUP):
                wps = pxwx.tile([P, 256], F32, tag="px")
                nc.tensor.matmul(wps, wu[0:8, 0:128], wu[0:8, :],
                                 start=True, stop=True)

            # ---- gram + K' = exp((S - sn/2)*c2), layout [n_chunk, t];
            # the -sn/2 bias rides in row 31 of each xT group (x the ones
            # row on the test side), so exps batch over chunk pairs ----
            kp = sb.tile([P, NK, TS], F32R)
            for cp in range(NK // 2):
                sg = pgram.tile([P, 2, TS], F32, tag="sg")
                for i in range(2):
                    c = 2 * cp + i
                    cc, g = c // 4, c % 4
                    nc.tensor.matmul(
                        sg[:, i, :],
                        xT[32 * g:32 * g + DP, ts(cc, P)],
                        xT[32 * g:32 * g + DP, 4 * P:],
                        start=True, stop=True,
                        tile_position=(32 * g, 0),
                    )
                nc.scalar.activation(
                    kp[:, 2 * cp:2 * cp + 2, :], sg,
                    mybir.ActivationFunctionType.Exp,
                    scale=c2,
                )

            # ---- per-half: XWX/XWy matmuls, assembly, solve, predict ----
            ga = sb.tile([P, NT, DP, W], F32)
            invp = sb.tile([P, NT, DP], F32)
            xsol = sb.tile([P, NT, DP], F32)
            yp = sb.tile([P, NT], F32)
            psr = [sb.tile([P, 2, DP], F32, name=f"psr{i}")
                   for i in range(2)]
            tbD = [sb.tile([P, 2, BAND, DP], F32, name=f"tbD{i}")
                   for i in range(2)]
            tbP = [sb.tile([P, 2, DP, 16], F32, name=f"tbP{i}")
                   for i in range(2)]
            bsc = [sb.tile([P, 2, D], F32, name=f"bsc{i}")
                   for i in range(2)]
            prod = [sb.tile([P, 2, DP], F32, name=f"prod{i}")
                   for i in range(2)]
            ga_diag = ga[:].rearrange("p b r c -> p b (r c)")[:, :, ::W + 1]

            for h in range(2):
                b0, b1 = 2 * h, 2 * h + 2
                # XWX | XWy: [256, 1056] per half via K'-chunks @ Z
                for t in range(b0, b1):
                    for (c0, c1) in ((0, 512), (512, NZ)):
                        w = c1 - c0
                        px = pxwx.tile([P, 512], F32, tag="px")
                        for c in range(NK):
                            nc.tensor.matmul(
                                px[:, :w],
                                kp[:, c, ts(t, P)],
                                zz[:, c, c0:c1],
                                start=(c == 0), stop=(c == NK - 1),
                            )
                        if c0 == 0:
                            # cols e=16..31, all rows d
                            nc.scalar.copy(
                                ga[:, t, :, H:DP],
                                px[:, :w].rearrange("p (r c) -> p r c", r=DP),
                            )
                        else:
                            # top-left quadrant + rhs column
                            nc.scalar.copy(
                                ga[:, t, 0:H, 0:H],
                                px[:, 0:H * H].rearrange(
                                    "p (r c) -> p r c", r=H),
                            )
                            nc.scalar.copy(
                                ga[:, t, :, DP], px[:, H * H:H * H + DP])

                # per-test ridge on the diagonal
                nc.gpsimd.tensor_add(
                    ga_diag[:, b0:b1], ga_diag[:, b0:b1],
                    regt[:, b0:b1, None].broadcast_to([P, 2, DP]),
                )
                # lower-left quadrant is never written by the evacuations
                # (symmetric solve reads upper only) but band rects read
                # across it; zero it so values stay finite
                nc.gpsimd.memset(ga[:, b0:b1, H:DP, 0:H], 0.0)

                # ---- symmetric (LDLt-style) elimination: A[i,k] = A[k,i],
                # so the update is upd[i,j] = ps[i]*prow[j] with ps = prow
                # *invp, and only the upper triangle + rhs is ever read.
                # DVE keeps a BAND-row window at the pivot (refilled every
                # BAND steps); Pool updates the remaining upper triangle as
                # column-panel rectangles.
                for k in range(DP):
                    nc.vector.reciprocal(
                        invp[:, b0:b1, k], ga[:, b0:b1, k, k])
                    if k == DP - 1:
                        break
                    m = D - k                  # rows k+1..31
                    w = W - 1 - k              # cols k+1..32 (incl rhs)
                    mD = min(BAND - (k % BAND), m)
                    r1 = k + 1 + mD            # first Pool row
                    prow = ga[:, b0:b1, k:k + 1, k + 1:W]
                    ps = psr[h]
                    nc.vector.tensor_mul(
                        ps[:, :, :w], ga[:, b0:b1, k, k + 1:W],
                        invp[:, b0:b1, k:k + 1].broadcast_to([P, 2, w]),
                    )
                    # DVE band: rows k+1..r1, all cols k+1..33
                    nc.vector.tensor_mul(
                        tbD[h][:, :, :mD, :w],
                        ps[:, :, :mD, None].broadcast_to([P, 2, mD, w]),
                        prow.broadcast_to([P, 2, mD, w]),
                    )
                    nc.vector.tensor_sub(
                        ga[:, b0:b1, k + 1:r1, k + 1:W],
                        ga[:, b0:b1, k + 1:r1, k + 1:W],
                        tbD[h][:, :, :mD, :w],
                    )
                    # Pool: upper-triangle rows r1..31 by column panels
                    for p in range(r1 // 8, 4):
                        re_ = min(8 * p + 8, DP)
                        if re_ <= r1:
                            continue
                        cs = max(8 * p, k + 1)
                        ce = W if p == 3 else 8 * p + 8
                        mr, wc = re_ - r1, ce - cs
                        nc.gpsimd.tensor_mul(
                            tbP[h][:, :, :mr, :wc],
                            ps[:, :, r1 - k - 1:re_ - k - 1, None]
                            .broadcast_to([P, 2, mr, wc]),
                            ga[:, b0:b1, k:k + 1, cs:ce]
                            .broadcast_to([P, 2, mr, wc]),
                        )
                        nc.gpsimd.tensor_sub(
                            ga[:, b0:b1, r1:re_, cs:ce],
                            ga[:, b0:b1, r1:re_, cs:ce],
                            tbP[h][:, :, :mr, :wc],
                        )

                # ---- backward substitution on the rhs column (Pool) ----
                for k in range(DP - 1, -1, -1):
                    nc.gpsimd.tensor_mul(
                        xsol[:, b0:b1, k], ga[:, b0:b1, k, DP],
                        invp[:, b0:b1, k],
                    )
                    if k == 0:
                        break
                    nc.gpsimd.tensor_mul(
                        bsc[h][:, :, :k],
                        ga[:, b0:b1, 0:k, k],
                        xsol[:, b0:b1, k:k + 1].broadcast_to([P, 2, k]),
                    )
                    nc.gpsimd.tensor_sub(
                        ga[:, b0:b1, 0:k, DP], ga[:, b0:b1, 0:k, DP],
                        bsc[h][:, :, :k],
                    )

                # ---- predictions: ypred = xtild_test . beta ----
                nc.gpsimd.tensor_mul(
                    prod[h], xtt[:, b0:b1], xsol[:, b0:b1])
                nc.vector.tensor_reduce(
                    yp[:, b0:b1], prod[h],
                    mybir.AxisListType.X, mybir.AluOpType.add,
                )

            nc.sync.dma_start(
                out_d.rearrange("(t p) one -> p (t one)", p=P), yp
            )

    nc.finalize()
    return nc


_cache: dict[float, object] = {}


def _get_nc(c2: float):
    if c2 not in _cache:
        _cache[c2] = _build_nc(c2)
    return _cache[c2]


def _build_xT(Xtrain, shard):
    """Pack [XtrT | XteT] with chunks at partition offsets 32g for 4-way
    row-group gram matmuls.  Row 31 of each group carries -|x|^2/2 on the
    train side and 1.0 on the test side, so the gram matmul computes
    S - sn/2 directly (no separate exp bias)."""
    out = np.zeros((P, 4 * P + TS), np.float32)
    XtrT = Xtrain.T
    nsn2 = -0.5 * np.sum(Xtrain * Xtrain, axis=1)       # [2048]
    for g in range(4):
        for cc in range(4):
            c = 4 * cc + g
            out[32 * g:32 * g + D, cc * P:(cc + 1) * P] = \
                XtrT[:, c * P:(c + 1) * P]
            out[32 * g + D, cc * P:(cc + 1) * P] = nsn2[c * P:(c + 1) * P]
        out[32 * g:32 * g + D, 4 * P:] = shard.T
        out[32 * g + D, 4 * P:] = 1.0
    return out


def _host_pack(Ytrain, Xtrain):
    """Train-side packing shared by all cores: the Z expansion."""
    Xt = np.concatenate(
        [np.ones((N_TRAIN, 1), np.float32), Xtrain], axis=1)  # [2048, 32]
    A = (Xt[:, :, None] * Xt[:, None, H:DP]).reshape(N_TRAIN, DP * H)
    B = (Xt[:, :H, None] * Xt[:, None, :H]).reshape(N_TRAIN, H * H)
    C = Xt * Ytrain[:, 0:1]
    zz = np.concatenate([A, B, C], axis=1)              # [2048, 800]
    return np.ascontiguousarray(
        zz.reshape(NK, P, NZ).transpose(1, 0, 2).reshape(P, NK * NZ))


def _host_pack_test(shard, c2):
    """Test-side packing per core: ridge scale + design rows."""
    st = np.sum(shard * shard, axis=1)                  # [512]
    regt = np.ascontiguousarray(
        (REG * np.exp(0.5 * c2 * st)).reshape(NT, P).T.astype(np.float32))
    xtt = np.concatenate(
        [np.ones((TS, 1), np.float32), shard], axis=1)  # [512, 32]
    xtt = np.ascontiguousarray(
        xtt.reshape(NT, P, DP).transpose(1, 0, 2).reshape(P, NT * DP))
    return regt, xtt


def kernel(Ytrain, Xtrain, Xtest, log_lengthscale, _trace=False):
    Ytrain = np.ascontiguousarray(np.asarray(Ytrain, dtype=np.float32))
    Xtrain = np.ascontiguousarray(np.asarray(Xtrain, dtype=np.float32))
    Xtest = np.ascontiguousarray(np.asarray(Xtest, dtype=np.float32))
    lls = float(np.asarray(log_lengthscale, dtype=np.float32))
    c2 = float(np.exp(np.float32(-2.0 * lls)))

    nc = _get_nc(c2)
    zz = _host_pack(Ytrain, Xtrain)
    in_maps = []
    for core in range(NCORES):
        shard = np.ascontiguousarray(Xtest[core * TS:(core + 1) * TS])
        regt, xtt = _host_pack_test(shard, c2)
        in_maps.append({
            "xT": _build_xT(Xtrain, shard),
            "zz": zz,
            "regt": regt,
            "xtt": xtt,
        })
    res = run_bass_kernel_spmd(nc, in_maps, list(range(NCORES)),
                               trace=bool(_trace))
    outs = [np.asarray(res.results[c]["ypred"], dtype=np.float32)
            for c in range(NCORES)]
    full = np.concatenate(outs, axis=0)
    if _trace:
        return full, res
    return full


def _sim_in_map(inputs):
    """Core-0 input map for CoreSim timing (test.py helper)."""
    Ytrain = np.asarray(inputs["Ytrain"], dtype=np.float32)
    Xtrain = np.asarray(inputs["Xtrain"], dtype=np.float32)
    Xtest = np.asarray(inputs["Xtest"], dtype=np.float32)
    lls = float(np.asarray(inputs["log_lengthscale"], dtype=np.float32))
    c2 = float(np.exp(np.float32(-2.0 * lls)))
    shard = np.ascontiguousarray(Xtest[:TS])
    zz = _host_pack(Ytrain, Xtrain)
    regt, xtt = _host_pack_test(shard, c2)
    return c2, {
        "xT": _build_xT(Xtrain, shard),
        "zz": zz,
        "regt": regt,
        "xtt": xtt,
    }


# revision 53
# speedup vs baseline: 1.6868x; 1.1982x over previous
"""Trainium2 Bass kernel: batched locally-weighted ridge regression.

Per test point t: K[t,n] = exp(-|xte_t - xtr_n|^2 / (2 ls^2));
  A_t = Xtild^T diag(K[t]) Xtild + REG*I ; b_t = Xtild^T (K[t] * Y)
  ypred_t = xtild_t . A_t^{-1} b_t
Sharding: data-parallel over the 4096 test points -> 8 cores x 512.

On-device math uses a scaled kernel K'[t,n] = exp((S[n,t] - sn[n]/2) * c2)
(c2 = 1/ls^2), i.e. the exp(-st*c2/2) per-test factor is dropped; this
rescales A_t and b_t identically, so beta is preserved by using a
per-test ridge REG_t = REG * exp(st*c2/2).

v2 layout:
  - Host precomputes the outer-product expansion Z = [x_d*x_e | x*y]
    (train-only data), squared-norm exp biases, per-test ridge, and the
    test design rows; DMA'd in parallel streams.
  - PE: warmup matmuls (clock ramp), gram S = XtrT-groups @ XteT in
    f32r, then XWX/XWy accumulation K'-chunks @ Z in f32r.
  - ACT: exp(S*c2 + bias), PSUM evacuations into [A|b] systems, mirror.
  - DVE+Pool: batched Gaussian elimination, 2 halves of 2x128 systems,
    rows of each update split DVE (top) / Pool (bottom); back-subst
    and prediction dot products on Pool; reductions/reciprocals on DVE.
"""

import numpy as np

import concourse.bacc as bacc
import concourse.mybir as mybir
from concourse.bass import ds, ts
from concourse.bass_utils import run_bass_kernel_spmd
from concourse.tile import TileContext

F32 = mybir.dt.float32
F32R = mybir.dt.float32r
P = 128
N_TRAIN = 2048
D = 31
DP = 32          # 1 + D
W = 34           # DP + two rhs columns: b (col 32), xtt (col 33)
N_TEST = 4096
NCORES = 8
TS = N_TEST // NCORES   # 512 test points per core
NT = TS // P            # 4 t-tiles
NK = N_TRAIN // P       # 16 train chunks
NZ = 672                # 4 e-strips of the upper pairs (64+128+192+256) + 32 (x*y)
REG = 1e-6
H = 16

N_WARMUP = 2            # PE clock-ramp warmup matmuls (fp32, 4-pass)
BAND = 10               # DVE pivot-band height (refill period)
UNITS = ((0, 1), (1, 2), (2, 3), (3, 4))   # elimination units


def _build_nc(c2: float):
    """Build the single-core Bass program (SPMD across 8 cores)."""
    nc = bacc.Bacc(trn_type="TRN2")

    # transposed features packed for 4-way row-group gram matmuls:
    # [32g+d, cc*128+p] = Xtrain[(4*cc+g)*128+p, d]; cols 512: = XtestT x4
    xT_d = nc.dram_tensor("xT", [P, 4 * P + TS], F32R, kind="ExternalInput")
    zz_d = nc.dram_tensor("zz", [P, NK * NZ], F32R, kind="ExternalInput")
    regt_d = nc.dram_tensor("regt", [P, NT], F32, kind="ExternalInput")
    xtt_d = nc.dram_tensor("xtt", [P, NT * DP], F32, kind="ExternalInput")
    out_d = nc.dram_tensor("ypred", [TS, 1], F32, kind="ExternalOutput")

    with TileContext(nc) as tc:
        with (
            tc.tile_pool(name="sb", bufs=1) as sb,
            tc.tile_pool(name="pgram", bufs=2, space="PSUM") as pgram,
            tc.tile_pool(name="pxwx", bufs=4, space="PSUM") as pxwx,
        ):
            # ---- input loads; zz spread across engine DMA queues so the
            # transfers run in parallel (issuing engine is charged the
            # transfer time in the DGE model)
            xT = sb.tile([P, 4 * P + TS], F32R)
            # need-first order: gram chunk c only reads train block c//4
            # plus the test columns, so land those before the rest
            nc.sync.dma_start(xT[:, 512:], xT_d[:, 512:])
            nc.sync.dma_start(xT[:, 0:P], xT_d[:, 0:P])
            nc.sync.dma_start(xT[:, P:512], xT_d[:, P:512])

            zz = sb.tile([P, NK, NZ], F32R)
            zr = zz[:].rearrange("p c z -> p (c z)")
            zq = [nc.sync, nc.gpsimd, nc.sync, nc.gpsimd,
                  nc.sync, nc.gpsimd, nc.sync, nc.gpsimd]
            for q in range(8):
                zq[q].dma_start(
                    zr[:, ts(q, 2 * NZ)], zz_d[:, ts(q, 2 * NZ)]
                )
            regt = sb.tile([P, NT], F32)
            nc.sync.dma_start(regt, regt_d[:, :])
            xtt = sb.tile([P, NT, DP], F32)
            nc.sync.dma_start(
                xtt, xtt_d.rearrange("p (t d) -> p t d", t=NT)
            )

            # ---- PE warmup: ramp the clock during the DMAs ----
            wu = sb.tile([P, 256], F32)
            nc.vector.memset(wu, 1.0)
            for _ in range(N_WARMUP):
                wps = pxwx.tile([P, 256], F32, tag="px")
                nc.tensor.matmul(wps, wu[0:8, 0:128], wu[0:8, :],
                                 start=True, stop=True)

            # ---- gram + K' = exp((S - sn/2)*c2), layout [n_chunk, t];
            # the -sn/2 bias rides in row 31 of each xT group (x the ones
            # row on the test side), so exps batch over chunk pairs ----
            kp = sb.tile([P, NK, TS], F32R)
            for cp in range(NK // 2):
                sg = pgram.tile([P, 2, TS], F32, tag="sg")
                for i in range(2):
                    c = 2 * cp + i
                    cc, g = c // 4, c % 4
                    nc.tensor.matmul(
                        sg[:, i, :],
                        xT[32 * g:32 * g + DP, ts(cc, P)],
                        xT[32 * g:32 * g + DP, 4 * P:],
                        start=True, stop=True,
                        tile_position=(32 * g, 0),
                    )
                nc.scalar.activation(
                    kp[:, 2 * cp:2 * cp + 2, :], sg,
                    mybir.ActivationFunctionType.Exp,
                    scale=c2,
                )

            # ---- per-half: XWX/XWy matmuls, assembly, solve, predict ----
            ga = sb.tile([P, NT, DP, W], F32)
            invp = sb.tile([P, NT, DP], F32)
            yp = sb.tile([P, NT], F32)
            NU = len(UNITS)
            psr = [sb.tile([P, u1 - u0, W - 1], F32, name=f"psr{i}")
                   for i, (u0, u1) in enumerate(UNITS)]
            tbD = [sb.tile([P, u1 - u0, BAND, W - 1], F32, name=f"tbD{i}")
                   for i, (u0, u1) in enumerate(UNITS)]
            tbP = [sb.tile([P, u1 - u0, DP, 16], F32, name=f"tbP{i}")
                   for i, (u0, u1) in enumerate(UNITS)]
            prod = [sb.tile([P, u1 - u0, DP], F32, name=f"prod{i}")
                   for i, (u0, u1) in enumerate(UNITS)]
            ga_diag = ga[:].rearrange("p b r c -> p b (r c)")[:, :, ::W + 1]

            for h, (b0, b1) in enumerate(UNITS):
                nb = b1 - b0
                # strips leave the sub-diagonal region unwritten; the band
                # rects read across it, so zero it first (the evacuations
                # overwrite the upper part afterwards)
                nc.vector.memset(ga[:, b0:b1, 8:DP, 0:24], 0.0)
                # XWX | XWy via K'-chunks @ Z; Z holds the upper-pair
                # e-strips s: (d, e) for e in [8s,8s+8), d in [0,8s+8)
                for t in range(b0, b1):
                    for (c0, c1) in ((0, 384), (384, NZ)):
                        w = c1 - c0
                        px = pxwx.tile([P, 512], F32, tag="px")
                        for c in range(NK):
                            nc.tensor.matmul(
                                px[:, :w],
                                kp[:, c, ts(t, P)],
                                zz[:, c, c0:c1],
                                start=(c == 0), stop=(c == NK - 1),
                            )
                        if c0 == 0:
                            # strips 0..2 (t0's go to idle DVE: it has a
                            # PSUM port and nothing to do this early)
                            off = 0
                            for s in range(3):
                                rs = 8 * s + 8
                                src_ap = px[:, off:off + 8 * rs].rearrange(
                                    "p (r c) -> p r c", r=rs)
                                if t == 0:
                                    nc.vector.tensor_scalar_mul(
                                        ga[:, t, 0:rs, 8 * s:8 * s + 8],
                                        src_ap, 1.0)
                                else:
                                    nc.scalar.copy(
                                        ga[:, t, 0:rs, 8 * s:8 * s + 8],
                                        src_ap)
                                off += 8 * rs
                        else:
                            # strip 3 + rhs column
                            nc.scalar.copy(
                                ga[:, t, :, 24:32],
                                px[:, 0:256].rearrange(
                                    "p (r c) -> p r c", r=DP),
                            )
                            nc.scalar.copy(
                                ga[:, t, :, DP], px[:, 256:256 + DP])
                            # second rhs column: the test design row; after
                            # forward elimination col 33 holds L^-1 xtt and
                            # ypred = sum_k invp_k * col32_k * col33_k
                            nc.scalar.copy(ga[:, t, :, DP + 1], xtt[:, t])

                # per-test ridge on the diagonal
                nc.gpsimd.tensor_add(
                    ga_diag[:, b0:b1], ga_diag[:, b0:b1],
                    regt[:, b0:b1, None].broadcast_to([P, nb, DP]),
                )

                # ---- symmetric (LDLt-style) elimination: A[i,k] = A[k,i],
                # so the update is upd[i,j] = ps[i]*prow[j] with ps = prow
                # *invp, and only the upper triangle + rhs is ever read.
                # DVE keeps a BAND-row window at the pivot (refilled every
                # BAND steps); Pool updates the remaining upper triangle as
                # column-panel rectangles.
                for k in range(DP):
                    nc.vector.reciprocal(
                        invp[:, b0:b1, k], ga[:, b0:b1, k, k])
                    if k == DP - 1:
                        break
                    m = D - k                  # rows k+1..31
                    w = W - 1 - k              # cols k+1..32 (incl rhs)
                    if m <= TAILM:
                        mD = m          # tail steps: skip the Pool split
                    else:
                        mD = min(BAND - (k % BAND), m)
                    r1 = k + 1 + mD            # first Pool row
                    prow = ga[:, b0:b1, k:k + 1, k + 1:W]
                    ps = psr[h]
                    if nb == 1:
                        # invp is a [P,1] per-partition scalar: fuse it into
                        # the band product on DVE (scalar_tensor_tensor) and
                        # let Pool build its own row factors (tensor_scalar)
                        iv1 = invp[:, b0, k:k + 1]
                        nc.vector.scalar_tensor_tensor(
                            tbD[h][:, 0, :mD, :w],
                            ga[:, b0, k, k + 1:r1, None]
                            .broadcast_to([P, mD, w]),
                            iv1,
                            ga[:, b0, k:k + 1, k + 1:W]
                            .broadcast_to([P, mD, w]),
                            op0=mybir.AluOpType.mult,
                            op1=mybir.AluOpType.mult,
                        )
                        if r1 < DP:
                            nc.gpsimd.tensor_scalar_mul(
                                ps[:, 0, :w],
                                ga[:, b0, k, k + 1:W], iv1,
                            )
                    else:
                        nc.vector.tensor_mul(
                            ps[:, :, :w], ga[:, b0:b1, k, k + 1:W],
                            invp[:, b0:b1, k:k + 1].broadcast_to([P, nb, w]),
                        )
                        nc.vector.tensor_mul(
                            tbD[h][:, :, :mD, :w],
                            ps[:, :, :mD, None].broadcast_to([P, nb, mD, w]),
                            prow.broadcast_to([P, nb, mD, w]),
                        )
                    # DVE band: rows k+1..r1, all cols k+1..33
                    nc.vector.tensor_sub(
                        ga[:, b0:b1, k + 1:r1, k + 1:W],
                        ga[:, b0:b1, k + 1:r1, k + 1:W],
                        tbD[h][:, :, :mD, :w],
                    )
                    # Pool: upper-triangle rows r1..31 by column panels
                    for p in range(r1 // 8, 4):
                        re_ = min(8 * p + 8, DP)
                        if re_ <= r1:
                            continue
                        cs = max(8 * p, k + 1)
                        ce = W if p == 3 else 8 * p + 8
                        mr, wc = re_ - r1, ce - cs
                        nc.gpsimd.tensor_mul(
                            tbP[h][:, :, :mr, :wc],
                            ps[:, :, r1 - k - 1:re_ - k - 1, None]
                            .broadcast_to([P, nb, mr, wc]),
                            ga[:, b0:b1, k:k + 1, cs:ce]
                            .broadcast_to([P, nb, mr, wc]),
                        )
                        nc.gpsimd.tensor_sub(
                            ga[:, b0:b1, r1:re_, cs:ce],
                            ga[:, b0:b1, r1:re_, cs:ce],
                            tbP[h][:, :, :mr, :wc],
                        )

                # ---- predictions: with A = L D L^T the forward pass
                # leaves c = L^-1 b in col 32 and u = L^-1 xtt in col 33;
                # ypred = sum_k u_k * c_k * invp_k (no back substitution)
                nc.gpsimd.tensor_mul(
                    prod[h], ga[:, b0:b1, :, DP], ga[:, b0:b1, :, DP + 1])
                nc.gpsimd.tensor_mul(
                    prod[h], prod[h], invp[:, b0:b1, :])
                nc.vector.tensor_reduce(
                    yp[:, b0:b1], prod[h],
                    mybir.AxisListType.X, mybir.AluOpType.add,
                )

            nc.sync.dma_start(
                out_d.rearrange("(t p) one -> p (t one)", p=P), yp
            )

    nc.finalize()
    return nc


_cache: dict[float, object] = {}


def _get_nc(c2: float):
    if c2 not in _cache:
        _cache[c2] = _build_nc(c2)
    return _cache[c2]


def _build_xT(Xtrain, shard):
    """Pack [XtrT | XteT] with chunks at partition offsets 32g for 4-way
    row-group gram matmuls.  Row 31 of each group carries -|x|^2/2 on the
    train side and 1.0 on the test side, so the gram matmul computes
    S - sn/2 directly (no separate exp bias)."""
    out = np.zeros((P, 4 * P + TS), np.float32)
    XtrT = Xtrain.T
    nsn2 = -0.5 * np.sum(Xtrain * Xtrain, axis=1)       # [2048]
    for g in range(4):
        for cc in range(4):
            c = 4 * cc + g
            out[32 * g:32 * g + D, cc * P:(cc + 1) * P] = \
                XtrT[:, c * P:(c + 1) * P]
            out[32 * g + D, cc * P:(cc + 1) * P] = nsn2[c * P:(c + 1) * P]
        out[32 * g:32 * g + D, 4 * P:] = shard.T
        out[32 * g + D, 4 * P:] = 1.0
    return out


def _host_pack(Ytrain, Xtrain):
    """Train-side packing shared by all cores: the Z expansion as four
    e-strips of the upper outer-product pairs plus the x*y column."""
    Xt = np.concatenate(
        [np.ones((N_TRAIN, 1), np.float32), Xtrain], axis=1)  # [2048, 32]
    parts = []
    for s in range(4):
        rs = 8 * s + 8
        parts.append((Xt[:, :rs, None] * Xt[:, None, 8 * s:8 * s + 8])
                     .reshape(N_TRAIN, rs * 8))
    parts.append(Xt * Ytrain[:, 0:1])
    zz = np.concatenate(parts, axis=1)                  # [2048, 672]
    return np.ascontiguousarray(
        zz.reshape(NK, P, NZ).transpose(1, 0, 2).reshape(P, NK * NZ))


def _host_pack_test(shard, c2):
    """Test-side packing per core: ridge scale + design rows."""
    st = np.sum(shard * shard, axis=1)                  # [512]
    regt = np.ascontiguousarray(
        (REG * np.exp(0.5 * c2 * st)).reshape(NT, P).T.astype(np.float32))
    xtt = np.concatenate(
        [np.ones((TS, 1), np.float32), shard], axis=1)  # [512, 32]
    xtt = np.ascontiguousarray(
        xtt.reshape(NT, P, DP).transpose(1, 0, 2).reshape(P, NT * DP))
    return regt, xtt


def kernel(Ytrain, Xtrain, Xtest, log_lengthscale, _trace=False):
    Ytrain = np.ascontiguousarray(np.asarray(Ytrain, dtype=np.float32))
    Xtrain = np.ascontiguousarray(np.asarray(Xtrain, dtype=np.float32))
    Xtest = np.ascontiguousarray(np.asarray(Xtest, dtype=np.float32))
    lls = float(np.asarray(log_lengthscale, dtype=np.float32))
    c2 = float(np.exp(np.float32(-2.0 * lls)))

    nc = _get_nc(c2)
    zz = _host_pack(Ytrain, Xtrain)
    in_maps = []
    for core in range(NCORES):
        shard = np.ascontiguousarray(Xtest[core * TS:(core + 1) * TS])
        regt, xtt = _host_pack_test(shard, c2)
        in_maps.append({
            "xT": _build_xT(Xtrain, shard),
            "zz": zz,
            "regt": regt,
            "xtt": xtt,
        })
    res = run_bass_kernel_spmd(nc, in_maps, list(range(NCORES)),
                               trace=bool(_trace))
    outs = [np.asarray(res.results[c]["ypred"], dtype=np.float32)
            for c in range(NCORES)]
    full = np.concatenate(outs, axis=0)
    if _trace:
        return full, res
    return full


def _sim_in_map(inputs):
    """Core-0 input map for CoreSim timing (test.py helper)."""
    Ytrain = np.asarray(inputs["Ytrain"], dtype=np.float32)
    Xtrain = np.asarray(inputs["Xtrain"], dtype=np.float32)
    Xtest = np.asarray(inputs["Xtest"], dtype=np.float32)
    lls = float(np.asarray(inputs["log_lengthscale"], dtype=np.float32))
    c2 = float(np.exp(np.float32(-2.0 * lls)))
    shard = np.ascontiguousarray(Xtest[:TS])
    zz = _host_pack(Ytrain, Xtrain)
    regt, xtt = _host_pack_test(shard, c2)
    return c2, {
        "xT": _build_xT(Xtrain, shard),
        "zz": zz,
        "regt": regt,
        "xtt": xtt,
    }
